# revision 19
# baseline (speedup 1.0000x reference)
"""Causal attention (B=4, S=4096, D=64, fp32) on 8 Trainium2 NeuronCores.

Strategy
--------
Sharding: 2 cores per batch element; the two cores of a batch split the KV
blocks by parity (even / odd 128-row blocks). Each core computes, for every
query position of its batch, the *unnormalized* attention numerator and the
softmax denominator contribution of its own KV half. The host sums the two
halves and divides (exactly linear: no max-subtraction; scores/8 are ~N(0,1)
so exp(s/8) <= ~1.7e3 stays in fp16 range; V and the denominator ones-column
are pre-scaled by 1/16 on the host - an exact power of two that cancels in
num/den - to keep the fp16 staging small).

Per-core device kernel (identical SPMD program; per-core behavior comes only
from input data), per 512-wide q tile, per kv block *pair*:
  - scores^T: S_T[kv, q] = K @ Q^T in fp16, two row-tiled matmuls (Q^T/K^T
    duplicated onto partitions 64-127 so the pair runs concurrently in the
    128x128 PE array), fp32 PSUM [128, 1024] (boundary pairs: 768 wide).
  - P = exp(s/8) in fp16, produced by TWO engines in parallel (the kernel is
    exp-throughput-bound; ACT alone was the baseline bottleneck):
      * ACT: activation(Exp, scale=0.125), PSUM -> fp16 SBUF,
        ~(w+172)/1.2GHz per pair.
      * DVE: one tensor_scalar op computing the Schraudolph bit trick
        bits = rne(s*(1024*log2e/8) + 15316) -> int16 SBUF, bitcast fp16
        (~+-3% sawtooth error; washes out after softmax normalization to
        ~4e-3 max rel err vs the 2e-2 budget), ~(w+~385)/0.96GHz.
    Pairs (and the per-tile output copies) are greedily load-balanced
    between the two engines with measured per-op costs.
  - causal masking: only the tile's diagonal (boundary) pair needs it; 0/1
    multiplicative masks are applied to P *after* exp on the otherwise-idle
    GPSIMD engine (keeps both exp engines and the PE mask-free). Boundary
    pairs run FIRST within their tile so the gpsimd latency hides behind the
    other pairs' exp stream (except the very first tile, where there is no
    backlog yet - there they run last).
  - numerator+denominator: matmul(lhsT=[V/16 | 1/16] block [128,65], rhs=P
    block [128,512]) accumulated over kv blocks in PSUM; row 64 is the
    softmax denominator. Padded keys: V rows and ones entries zeroed on host.
    mm2s are emitted TWO pairs behind their exp (software pipelining) so the
    in-order PE queue never parks on an exp semaphore while mm1 work exists -
    PE idle gaps would re-arm the HAM clock throttle and halve the PE clock.
  - output: [65, 512] PSUM -> fp16 SBUF copy (engine greedy) -> DMA.
Host: transposes Q/K, packs per-core inputs, combines/normalizes/transposes.
"""

import numpy as np
from contextlib import ExitStack

import concourse.tile as tile
from concourse import bacc, mybir
from concourse.bass_utils import run_bass_kernel_spmd

B, S, D = 4, 4096, 64
NCORES = 8
BLK = 128            # kv block rows
QTW = 512            # q tile width
NQT = S // QTW       # 8 q tiles
PAR = S // BLK // 2  # 16 kv blocks per parity half
WARMUP_MMS = 9       # dummy matmuls to open the PE HAM clock gate at startup
DEFER = 4            # pairs of software-pipelining between exp and mm2

LOG2E = float(np.log2(np.e))
SCHR_A = 1024.0 * LOG2E / 8.0   # d(bits)/d(score) for fp16 exp(s/8)
SCHR_B = 15360.0 - 44.0         # fp16 exponent bias + sawtooth centering

# Measured per-op engine costs (ns) for the greedy balance.
COST_ACT = {1024: 997.0, 768: 783.0}
COST_DVE = {1024: 1230.0, 768: 960.0}
COST_STT = 950.0     # boundary scalar_tensor_tensor (768 wide) on DVE
COST_COPY = 1110.0   # merged [65, 1024] fp32->fp16 PSUM->SBUF copy
# Tiles are processed in this order; consecutive tiles SHARE one [65, 1024]
# PSUM accumulator (out_ps bufs=1) so there are 4 output copies, not 8.
TILE_ORDER = [7, 0, 6, 5, 4, 3, 2, 1]

_prog_cache = {}


def _schedule():
    """[(T, p, boundary, engine)] in processing order + per-group copy engine.
    engine/copy: 0 = ACT, 1 = DVE. Groups are TILE_ORDER[2g:2g+2]."""
    seq = []
    for ti, T in enumerate(TILE_ORDER):
        npair = T + 1
        body = list(range(npair - 1))
        pairs = body + [npair - 1] if ti == 0 else [npair - 1] + body
        for p in pairs:
            seq.append((T, p, p == npair - 1))
    t_eng = [0.0, 0.0]
    out = []
    copy_eng = {}
    done_pairs = {T: 0 for T in TILE_ORDER}
    group_of = {T: gi // 2 for gi, T in enumerate(TILE_ORDER)}
    group_left = {g: 0 for g in range(len(TILE_ORDER) // 2)}
    for T in TILE_ORDER:
        group_left[group_of[T]] += T + 1
    for T, p, bnd in seq:
        if bnd:
            # boundary pairs carry the causal mask fused into the DVE
            # scalar_tensor_tensor op - DVE only
            t_eng[1] += COST_STT
            out.append((T, p, bnd, 1))
            done_pairs[T] += 1
            group_left[group_of[T]] -= 1
            if group_left[group_of[T]] == 0:
                e = 0 if t_eng[0] <= t_eng[1] else 1
                t_eng[e] += COST_COPY
                copy_eng[group_of[T]] = e
            continue
        ca, cv = COST_ACT[1024], COST_DVE[1024]
        if t_eng[0] + ca <= t_eng[1] + cv:
            t_eng[0] += ca
            out.append((T, p, bnd, 0))
        else:
            t_eng[1] += cv
            out.append((T, p, bnd, 1))
        done_pairs[T] += 1
        group_left[group_of[T]] -= 1
        if group_left[group_of[T]] == 0:  # group complete -> assign its copy
            e = 0 if t_eng[0] <= t_eng[1] else 1
            t_eng[e] += COST_COPY
            copy_eng[group_of[T]] = e
    return out, copy_eng


def _build_program():
    if "nc" in _prog_cache:
        return _prog_cache["nc"]
    nc = bacc.Bacc("TRN2", target_bir_lowering=False, debug=False, num_devices=NCORES)
    f32, f16, i16 = mybir.dt.float32, mybir.dt.float16, mybir.dt.int16
    Exp = mybir.ActivationFunctionType.Exp
    TWO = QTW * 2

    qt_d = nc.dram_tensor("qt", [2 * D, S], f16, kind="ExternalInput").ap()
    kt_d = nc.dram_tensor("kt", [2 * D, PAR * BLK], f16, kind="ExternalInput").ap()
    vp_d = nc.dram_tensor("vp", [BLK, PAR * BLK], f16, kind="ExternalInput").ap()
    mk_d = nc.dram_tensor("mk", [BLK, QTW + QTW // 2], f16, kind="ExternalInput").ap()
    out_d = nc.dram_tensor("out", [65, S], f16, kind="ExternalOutput").ap()

    sched, copy_eng = _schedule()

    with tile.TileContext(nc) as tc, ExitStack() as ctx:
        const = ctx.enter_context(tc.tile_pool(name="const", bufs=1))
        pa_pool = ctx.enter_context(tc.tile_pool(name="pa", bufs=5))
        pv_pool = ctx.enter_context(tc.tile_pool(name="pv", bufs=5))
        opool = ctx.enter_context(tc.tile_pool(name="op", bufs=2))
        sc_ps = ctx.enter_context(tc.tile_pool(name="scps", bufs=3, space="PSUM"))
        out_ps = ctx.enter_context(tc.tile_pool(name="ops", bufs=1, space="PSUM"))

        # Input DMAs spread over three rings (sync HWDGE, scalar HWDGE,
        # gpsimd SWDGE) in first-use order (tile 7, kv pairs ascending).
        mk_s = const.tile([BLK, QTW + QTW // 2], f16)
        kt_s = const.tile([2 * D, PAR * BLK], f16)
        vp_s = const.tile([BLK, PAR * BLK], f16)
        qt_s = const.tile([2 * D, S], f16)
        nc.scalar.dma_start(kt_s[:, 0:256], kt_d[:, 0:256])
        nc.gpsimd.dma_start(vp_s[:], vp_d[:])
        nc.scalar.dma_start(kt_s[:, 256:512], kt_d[:, 256:512])
        nc.scalar.dma_start(kt_s[:, 512:1024], kt_d[:, 512:1024])
        nc.scalar.dma_start(kt_s[:, 1024:1536], kt_d[:, 1024:1536])
        nc.scalar.dma_start(kt_s[:, 1536:], kt_d[:, 1536:])
        nc.gpsimd.dma_start(mk_s[:], mk_d[:])
        for t in [7, 0, 6, 5, 4, 3, 2, 1]:  # matches tile processing order
            nc.sync.dma_start(qt_s[:, t * QTW : (t + 1) * QTW], qt_d[:, t * QTW : (t + 1) * QTW])

        # PE warmup: dependency-free dummy matmuls during the preamble/DMA
        # window so the HAM clock gate (PE parked at 1.2 GHz until ~3.4us of
        # busy) opens before the first real matmul.
        wsrc = const.tile([BLK, QTW], f16, name="wsrc")
        nc.vector.memset(wsrc[:], 0.0)
        wps = sc_ps.tile([BLK, TWO], f32, tag="sc", name="wps")
        for _ in range(WARMUP_MMS):
            nc.tensor.matmul(wps[:, 0:QTW], wsrc[:, 0:BLK], wsrc[:], start=True, stop=True)

        # ---- software-pipelined main loop ----
        group_of = {T: gi // 2 for gi, T in enumerate(TILE_ORDER)}
        col_of = {T: (gi % 2) * QTW for gi, T in enumerate(TILE_ORDER)}
        gstate = {}   # group -> dict(ops=..., n per tile)
        pending = []  # deferred mm2 work items

        last_group = len(TILE_ORDER) // 2 - 1

        def emit_mm2(item):
            T, p, boundary, pt, wid = item
            g = group_of[T]
            st = gstate[g]
            depth = 2 * (T + 1)
            off = col_of[T]
            for k in (0, 1):
                blk = 2 * p + k
                st[T] += 1
                nc.tensor.matmul(
                    st["ops"][0:BLK, off + QTW - wid[k] : off + QTW],
                    vp_s[:, blk * BLK : blk * BLK + BLK],
                    pt[0:BLK, k * QTW : k * QTW + wid[k]],
                    start=(st[T] == 1),
                    stop=(st[T] == depth),
                )
            st["left"] -= 1
            if g == last_group:
                # tail: copy/DMA each tile as soon as it completes, so only
                # the final tile's [65, 512] chain trails the last exp.
                if st[T] == depth:
                    osb = opool.tile([65, QTW], f16, tag="osb", name=f"osbt{T}")
                    nc.scalar.copy(osb[:], st["ops"][0:65, col_of[T] : col_of[T] + QTW])
                    nc.scalar.dma_start(out_d[:, T * QTW : (T + 1) * QTW], osb[:])
            elif st["left"] == 0:
                osb = opool.tile([65, TWO], f16, tag="osb", name=f"osb{g}")
                if copy_eng[g] == 0:
                    nc.scalar.copy(osb[:], st["ops"][0:65, :])
                else:
                    nc.vector.tensor_copy(osb[:], st["ops"][0:65, :])
                for TT in TILE_ORDER[2 * g : 2 * g + 2]:
                    nc.sync.dma_start(
                        out_d[:, TT * QTW : (TT + 1) * QTW],
                        osb[:, col_of[TT] : col_of[TT] + QTW],
                    )

        # Pairs are processed in batches of 2: both pairs' mm1s are emitted
        # back-to-back (row-tiled PE config), then the deferred mm2s (full
        # 128x128 config) - one config switch per direction per batch instead
        # of per pair. Each switch exposes one ~100ns weight load.
        for ci in range(0, len(sched), 2):
            chunk = sched[ci : ci + 2]
            exps = []
            for T, p, boundary, eng in chunk:
                g = group_of[T]
                if g not in gstate:
                    gstate[g] = {
                        "ops": out_ps.tile([BLK, TWO], f32, tag="ops", name=f"ops{g}"),
                        "left": sum(TT + 1 for TT in TILE_ORDER[2 * g : 2 * g + 2]),
                    }
                    for TT in TILE_ORDER[2 * g : 2 * g + 2]:
                        gstate[g][TT] = 0
                sc = sc_ps.tile([BLK, TWO], f32, tag="sc")
                wid = (QTW, QTW // 2) if boundary else (QTW, QTW)
                for k, rg in ((0, 0), (1, D)):  # row group 0 / 64 (row tiling)
                    blk = 2 * p + k
                    nc.tensor.matmul(
                        sc[:, k * QTW : k * QTW + wid[k]],
                        kt_s[rg : rg + D, blk * BLK : (blk + 1) * BLK],
                        qt_s[rg : rg + D, (T + 1) * QTW - wid[k] : (T + 1) * QTW],
                        start=True,
                        stop=True,
                        tile_position=(rg, 0),
                    )
                exps.append((T, p, boundary, eng, sc, wid))
            for T, p, boundary, eng, sc, wid in exps:
                ew = QTW + wid[1]
                if boundary:
                    # DVE fast-exp with the causal mask FUSED as an additive
                    # bias tensor: bits = rne(s*A + mb) -> int16, bitcast
                    # fp16. Masked: mb=-60000 saturates to -32768 = -0.0.
                    pt_raw = pv_pool.tile([BLK, TWO], i16, tag="pv", name="ptv")
                    nc.vector.scalar_tensor_tensor(
                        pt_raw[:, 0:ew],
                        sc[:, 0:ew],
                        SCHR_A,
                        mk_s[:, 0:ew],
                        mybir.AluOpType.mult,
                        mybir.AluOpType.add,
                    )
                    pt = pt_raw[:].bitcast(f16)
                elif eng == 1:
                    # DVE fast-exp: bits = rne(s*A+B) -> int16, bitcast fp16.
                    pt_raw = pv_pool.tile([BLK, TWO], i16, tag="pv", name="ptv")
                    nc.vector.tensor_scalar(
                        pt_raw[:, 0:ew],
                        sc[:, 0:ew],
                        SCHR_A,
                        SCHR_B,
                        mybir.AluOpType.mult,
                        mybir.AluOpType.add,
                    )
                    pt = pt_raw[:].bitcast(f16)
                else:
                    pt_raw = pa_pool.tile([BLK, TWO], f16, tag="pa", name="pta")
                    pt = pt_raw[:]
                    nc.scalar.activation(pt[0:BLK, 0:ew], sc[:, 0:ew], Exp, scale=0.125)
                pending.append((T, p, boundary, pt, wid))
            while len(pending) > DEFER:
                emit_mm2(pending.pop(0))
        while pending:
            emit_mm2(pending.pop(0))

    nc.compile()
    _prog_cache["nc"] = nc
    return nc


def _make_maskbias(h):
    """[128, 768] fp16 additive Schraudolph bias for the boundary pair:
    +B where kept, -60000 where masked (saturates the int16 convert to
    -32768 = fp16 -0.0). Cols 0:512 = lo block (relative diagonal offset
    r = h); cols 512:768 = the computed 256-col slice of the hi block
    (r = h + 2, its q cols 256:512)."""
    tri = np.arange(QTW)[None, :BLK] >= np.arange(BLK)[:, None]
    full = np.zeros((BLK, BLK), dtype=bool)
    keep = np.ones((BLK, BLK), dtype=bool)

    def keep_for_r(r):
        cols = []
        for cb in range(QTW // BLK):
            cols.append(full if cb < r else tri if cb == r else keep)
        return np.concatenate(cols, axis=1)  # [128, 512] bool

    kp = np.concatenate([keep_for_r(h), keep_for_r(h + 2)[:, QTW // 2 :]], axis=1)
    return np.where(kp, np.float16(SCHR_B), np.float16(-60000.0))


def kernel(query, key, value, padding):
    query = np.asarray(query, dtype=np.float32)
    key = np.asarray(key, dtype=np.float32)
    value = np.asarray(value, dtype=np.float32)
    padding = np.asarray(padding, dtype=bool)

    nc = _build_program()

    in_maps = []
    for c in range(NCORES):
        b, h = divmod(c, 2)
        qt1 = np.ascontiguousarray(query[b].T).astype(np.float16)  # [64, 4096]
        qt = np.concatenate([qt1, qt1], axis=0)  # [128, 4096] (row-tiling dup)
        kT = key[b].T  # [64, 4096] view
        blocks = [2 * i + h for i in range(PAR)]
        kt = np.concatenate([kT[:, BLK * j : BLK * (j + 1)] for j in blocks], axis=1)
        kt1 = np.ascontiguousarray(kt).astype(np.float16)  # [64, 2048]
        kt = np.concatenate([kt1, kt1], axis=0)  # [128, 2048] (row-tiling dup)
        vp = np.zeros((BLK, PAR * BLK), dtype=np.float16)
        for i, j in enumerate(blocks):
            vblk = value[b, BLK * j : BLK * (j + 1), :].copy()
            pblk = padding[b, BLK * j : BLK * (j + 1)]
            vblk[pblk] = 0.0
            # 1/16 scaling (exact power of two) keeps fp16 staging small;
            # cancels in num/den on the host.
            vp[:, BLK * i : BLK * i + 64] = vblk / 16.0
            vp[:, BLK * i + 64] = np.where(pblk, 0.0, 1.0 / 16.0)
        in_maps.append({"qt": qt, "kt": kt, "vp": vp, "mk": _make_maskbias(h)})

    global _last_in_maps
    _last_in_maps = in_maps
    res = run_bass_kernel_spmd(nc, in_maps, list(range(NCORES)))

    out = np.empty((B, S, D), dtype=np.float32)
    for b in range(B):
        r0 = res.results[2 * b]["out"].astype(np.float64)
        r1 = res.results[2 * b + 1]["out"].astype(np.float64)
        num = r0[:64] + r1[:64]  # [64, 4096]
        den = r0[64] + r1[64]  # [4096]
        out[b] = (num / den).T.astype(np.float32)
    return out


# revision 20
# speedup vs baseline: 1.0193x; 1.0193x over previous
"""Causal attention (B=4, S=4096, D=64, fp32) on 8 Trainium2 NeuronCores.

Strategy
--------
Sharding: 2 cores per batch element; the two cores of a batch split the KV
blocks by parity (even / odd 128-row blocks). Each core computes, for every
query position of its batch, the *unnormalized* attention numerator and the
softmax denominator contribution of its own KV half. The host sums the two
halves and divides (exactly linear: no max-subtraction; scores/8 are ~N(0,1)
so exp(s/8) <= ~1.7e3 stays in fp16 range; V and the denominator ones-column
are pre-scaled by 1/16 on the host - an exact power of two that cancels in
num/den - to keep the fp16 staging small).

Per-core device kernel (identical SPMD program; per-core behavior comes only
from input data), per 512-wide q tile, per kv block *pair*:
  - scores^T: S_T[kv, q] = K @ Q^T in fp16, two row-tiled matmuls (Q^T/K^T
    duplicated onto partitions 64-127 so the pair runs concurrently in the
    128x128 PE array), fp32 PSUM [128, 1024] (boundary pairs: 768 wide).
  - P = exp(s/8) in fp16, produced by TWO engines in parallel (the kernel is
    exp-throughput-bound; ACT alone was the baseline bottleneck):
      * ACT: activation(Exp, scale=0.125), PSUM -> fp16 SBUF,
        ~(w+172)/1.2GHz per pair.
      * DVE: one tensor_scalar op computing the Schraudolph bit trick
        bits = rne(s*(1024*log2e/8) + 15316) -> int16 SBUF, bitcast fp16
        (~+-3% sawtooth error; washes out after softmax normalization to
        ~4e-3 max rel err vs the 2e-2 budget), ~(w+~385)/0.96GHz.
    Pairs (and the per-tile output copies) are greedily load-balanced
    between the two engines with measured per-op costs.
  - causal masking: only the tile's diagonal (boundary) pair needs it; 0/1
    multiplicative masks are applied to P *after* exp on the otherwise-idle
    GPSIMD engine (keeps both exp engines and the PE mask-free). Boundary
    pairs run FIRST within their tile so the gpsimd latency hides behind the
    other pairs' exp stream (except the very first tile, where there is no
    backlog yet - there they run last).
  - numerator+denominator: matmul(lhsT=[V/16 | 1/16] block [128,65], rhs=P
    block [128,512]) accumulated over kv blocks in PSUM; row 64 is the
    softmax denominator. Padded keys: V rows and ones entries zeroed on host.
    mm2s are emitted TWO pairs behind their exp (software pipelining) so the
    in-order PE queue never parks on an exp semaphore while mm1 work exists -
    PE idle gaps would re-arm the HAM clock throttle and halve the PE clock.
  - output: [65, 512] PSUM -> fp16 SBUF copy (engine greedy) -> DMA.
Host: transposes Q/K, packs per-core inputs, combines/normalizes/transposes.
"""

import numpy as np
from contextlib import ExitStack

import concourse.tile as tile
from concourse import bacc, mybir
from concourse.bass_utils import run_bass_kernel_spmd

B, S, D = 4, 4096, 64
NCORES = 8
BLK = 128            # kv block rows
QTW = 512            # q tile width
NQT = S // QTW       # 8 q tiles
PAR = S // BLK // 2  # 16 kv blocks per parity half
WARMUP_MMS = 9       # dummy matmuls to open the PE HAM clock gate at startup
DEFER = 3            # pairs of software-pipelining between exp and mm2

LOG2E = float(np.log2(np.e))
SCHR_A = 1024.0 * LOG2E / 8.0   # d(bits)/d(score) for fp16 exp(s/8)
SCHR_B = 15360.0 - 44.0         # fp16 exponent bias + sawtooth centering

# Measured per-op engine costs (ns) for the greedy balance.
COST_ACT = {1024: 997.0, 768: 783.0}
COST_DVE = {1024: 1230.0, 768: 960.0}
COST_STT = 950.0     # boundary scalar_tensor_tensor (768 wide) on DVE
COST_COPY = 1110.0   # merged [65, 1024] fp32->fp16 PSUM->SBUF copy
# Tiles are processed in this order; consecutive tiles SHARE one [65, 1024]
# PSUM accumulator (out_ps bufs=1) so there are 4 output copies, not 8.
TILE_ORDER = [7, 0, 6, 5, 4, 3, 2, 1]

_prog_cache = {}


def _schedule():
    """[(T, p, boundary, engine)] in processing order + per-group copy engine.
    engine/copy: 0 = ACT, 1 = DVE. Groups are TILE_ORDER[2g:2g+2]."""
    seq = []
    for ti, T in enumerate(TILE_ORDER):
        npair = T + 1
        body = list(range(npair - 1))
        pairs = body + [npair - 1] if ti == 0 else [npair - 1] + body
        for p in pairs:
            seq.append((T, p, p == npair - 1))
    t_eng = [0.0, 0.0]
    out = []
    copy_eng = {}
    done_pairs = {T: 0 for T in TILE_ORDER}
    group_of = {T: gi // 2 for gi, T in enumerate(TILE_ORDER)}
    group_left = {g: 0 for g in range(len(TILE_ORDER) // 2)}
    for T in TILE_ORDER:
        group_left[group_of[T]] += T + 1
    for T, p, bnd in seq:
        if bnd:
            # boundary pairs carry the causal mask fused into the DVE
            # scalar_tensor_tensor op - DVE only
            t_eng[1] += COST_STT
            out.append((T, p, bnd, 1))
            done_pairs[T] += 1
            group_left[group_of[T]] -= 1
            if group_left[group_of[T]] == 0:
                e = 0 if t_eng[0] <= t_eng[1] else 1
                t_eng[e] += COST_COPY
                copy_eng[group_of[T]] = e
            continue
        ca, cv = COST_ACT[1024], COST_DVE[1024]
        if t_eng[0] + ca <= t_eng[1] + cv:
            t_eng[0] += ca
            out.append((T, p, bnd, 0))
        else:
            t_eng[1] += cv
            out.append((T, p, bnd, 1))
        done_pairs[T] += 1
        group_left[group_of[T]] -= 1
        if group_left[group_of[T]] == 0:  # group complete -> assign its copy
            e = 0 if t_eng[0] <= t_eng[1] else 1
            t_eng[e] += COST_COPY
            copy_eng[group_of[T]] = e
    return out, copy_eng


def _build_program():
    if "nc" in _prog_cache:
        return _prog_cache["nc"]
    nc = bacc.Bacc("TRN2", target_bir_lowering=False, debug=False, num_devices=NCORES)
    f32, f16, i16 = mybir.dt.float32, mybir.dt.float16, mybir.dt.int16
    Exp = mybir.ActivationFunctionType.Exp
    TWO = QTW * 2

    qt_d = nc.dram_tensor("qt", [2 * D, S], f16, kind="ExternalInput").ap()
    kt_d = nc.dram_tensor("kt", [2 * D, PAR * BLK], f16, kind="ExternalInput").ap()
    vp_d = nc.dram_tensor("vp", [BLK, PAR * BLK], f16, kind="ExternalInput").ap()
    mk_d = nc.dram_tensor("mk", [BLK, QTW + QTW // 2], f16, kind="ExternalInput").ap()
    out_d = nc.dram_tensor("out", [65, S], f16, kind="ExternalOutput").ap()

    sched, copy_eng = _schedule()

    with tile.TileContext(nc) as tc, ExitStack() as ctx:
        const = ctx.enter_context(tc.tile_pool(name="const", bufs=1))
        pa_pool = ctx.enter_context(tc.tile_pool(name="pa", bufs=5))
        pv_pool = ctx.enter_context(tc.tile_pool(name="pv", bufs=5))
        opool = ctx.enter_context(tc.tile_pool(name="op", bufs=2))
        sc_ps = ctx.enter_context(tc.tile_pool(name="scps", bufs=3, space="PSUM"))
        out_ps = ctx.enter_context(tc.tile_pool(name="ops", bufs=1, space="PSUM"))

        # Input DMAs spread over three rings (sync HWDGE, scalar HWDGE,
        # gpsimd SWDGE) in first-use order (tile 7, kv pairs ascending).
        mk_s = const.tile([BLK, QTW + QTW // 2], f16)
        kt_s = const.tile([2 * D, PAR * BLK], f16)
        vp_s = const.tile([BLK, PAR * BLK], f16)
        qt_s = const.tile([2 * D, S], f16)
        nc.scalar.dma_start(kt_s[:, 0:256], kt_d[:, 0:256])
        nc.gpsimd.dma_start(vp_s[:], vp_d[:])
        nc.scalar.dma_start(kt_s[:, 256:512], kt_d[:, 256:512])
        nc.scalar.dma_start(kt_s[:, 512:1024], kt_d[:, 512:1024])
        nc.scalar.dma_start(kt_s[:, 1024:1536], kt_d[:, 1024:1536])
        nc.scalar.dma_start(kt_s[:, 1536:], kt_d[:, 1536:])
        nc.gpsimd.dma_start(mk_s[:], mk_d[:])
        for t in [7, 0, 6, 5, 4, 3, 2, 1]:  # matches tile processing order
            nc.sync.dma_start(qt_s[:, t * QTW : (t + 1) * QTW], qt_d[:, t * QTW : (t + 1) * QTW])

        # PE warmup: dependency-free dummy matmuls during the preamble/DMA
        # window so the HAM clock gate (PE parked at 1.2 GHz until ~3.4us of
        # busy) opens before the first real matmul.
        wsrc = const.tile([BLK, QTW], f16, name="wsrc")
        nc.vector.memset(wsrc[:], 0.0)
        wps = sc_ps.tile([BLK, TWO], f32, tag="sc", name="wps")
        for _ in range(WARMUP_MMS):
            nc.tensor.matmul(wps[:, 0:QTW], wsrc[:, 0:BLK], wsrc[:], start=True, stop=True)

        # ---- software-pipelined main loop ----
        group_of = {T: gi // 2 for gi, T in enumerate(TILE_ORDER)}
        col_of = {T: (gi % 2) * QTW for gi, T in enumerate(TILE_ORDER)}
        gstate = {}   # group -> dict(ops=..., n per tile)
        pending = []  # deferred mm2 work items

        last_group = len(TILE_ORDER) // 2 - 1

        def emit_mm2(item):
            T, p, boundary, pt, wid = item
            g = group_of[T]
            st = gstate[g]
            depth = 2 * (T + 1)
            off = col_of[T]
            for k in (0, 1):
                blk = 2 * p + k
                st[T] += 1
                nc.tensor.matmul(
                    st["ops"][0:BLK, off + QTW - wid[k] : off + QTW],
                    vp_s[:, blk * BLK : blk * BLK + BLK],
                    pt[0:BLK, k * QTW : k * QTW + wid[k]],
                    start=(st[T] == 1),
                    stop=(st[T] == depth),
                )
            st["left"] -= 1
            if g == last_group:
                # tail: copy/DMA each tile as soon as it completes, so only
                # the final tile's [65, 512] chain trails the last exp.
                if st[T] == depth:
                    osb = opool.tile([65, QTW], f16, tag="osb", name=f"osbt{T}")
                    nc.scalar.copy(osb[:], st["ops"][0:65, col_of[T] : col_of[T] + QTW])
                    nc.scalar.dma_start(out_d[:, T * QTW : (T + 1) * QTW], osb[:])
            elif st["left"] == 0:
                osb = opool.tile([65, TWO], f16, tag="osb", name=f"osb{g}")
                if copy_eng[g] == 0:
                    nc.scalar.copy(osb[:], st["ops"][0:65, :])
                else:
                    nc.vector.tensor_copy(osb[:], st["ops"][0:65, :])
                for TT in TILE_ORDER[2 * g : 2 * g + 2]:
                    nc.sync.dma_start(
                        out_d[:, TT * QTW : (TT + 1) * QTW],
                        osb[:, col_of[TT] : col_of[TT] + QTW],
                    )

        # Pairs are processed in batches of 2: both pairs' mm1s are emitted
        # back-to-back (row-tiled PE config), then the deferred mm2s (full
        # 128x128 config) - one config switch per direction per batch instead
        # of per pair. Each switch exposes one ~100ns weight load.
        for ci in range(0, len(sched), 2):
            chunk = sched[ci : ci + 2]
            exps = []
            for T, p, boundary, eng in chunk:
                g = group_of[T]
                if g not in gstate:
                    gstate[g] = {
                        "ops": out_ps.tile([BLK, TWO], f32, tag="ops", name=f"ops{g}"),
                        "left": sum(TT + 1 for TT in TILE_ORDER[2 * g : 2 * g + 2]),
                    }
                    for TT in TILE_ORDER[2 * g : 2 * g + 2]:
                        gstate[g][TT] = 0
                sc = sc_ps.tile([BLK, TWO], f32, tag="sc")
                wid = (QTW, QTW // 2) if boundary else (QTW, QTW)
                for k, rg in ((0, 0), (1, D)):  # row group 0 / 64 (row tiling)
                    blk = 2 * p + k
                    nc.tensor.matmul(
                        sc[:, k * QTW : k * QTW + wid[k]],
                        kt_s[rg : rg + D, blk * BLK : (blk + 1) * BLK],
                        qt_s[rg : rg + D, (T + 1) * QTW - wid[k] : (T + 1) * QTW],
                        start=True,
                        stop=True,
                        tile_position=(rg, 0),
                    )
                exps.append((T, p, boundary, eng, sc, wid))
            for T, p, boundary, eng, sc, wid in exps:
                ew = QTW + wid[1]
                if boundary:
                    # DVE fast-exp with the causal mask FUSED as an additive
                    # bias tensor: bits = rne(s*A + mb) -> int16, bitcast
                    # fp16. Masked: mb=-60000 saturates to -32768 = -0.0.
                    pt_raw = pv_pool.tile([BLK, TWO], i16, tag="pv", name="ptv")
                    nc.vector.scalar_tensor_tensor(
                        pt_raw[:, 0:ew],
                        sc[:, 0:ew],
                        SCHR_A,
                        mk_s[:, 0:ew],
                        mybir.AluOpType.mult,
                        mybir.AluOpType.add,
                    )
                    pt = pt_raw[:].bitcast(f16)
                elif eng == 1:
                    # DVE fast-exp: bits = rne(s*A+B) -> int16, bitcast fp16.
                    pt_raw = pv_pool.tile([BLK, TWO], i16, tag="pv", name="ptv")
                    nc.vector.tensor_scalar(
                        pt_raw[:, 0:ew],
                        sc[:, 0:ew],
                        SCHR_A,
                        SCHR_B,
                        mybir.AluOpType.mult,
                        mybir.AluOpType.add,
                    )
                    pt = pt_raw[:].bitcast(f16)
                else:
                    pt_raw = pa_pool.tile([BLK, TWO], f16, tag="pa", name="pta")
                    pt = pt_raw[:]
                    nc.scalar.activation(pt[0:BLK, 0:ew], sc[:, 0:ew], Exp, scale=0.125)
                pending.append((T, p, boundary, pt, wid))
            while len(pending) > DEFER:
                emit_mm2(pending.pop(0))
        while pending:
            emit_mm2(pending.pop(0))

    nc.compile()
    _prog_cache["nc"] = nc
    return nc


def _make_maskbias(h):
    """[128, 768] fp16 additive Schraudolph bias for the boundary pair:
    +B where kept, -60000 where masked (saturates the int16 convert to
    -32768 = fp16 -0.0). Cols 0:512 = lo block (relative diagonal offset
    r = h); cols 512:768 = the computed 256-col slice of the hi block
    (r = h + 2, its q cols 256:512)."""
    tri = np.arange(QTW)[None, :BLK] >= np.arange(BLK)[:, None]
    full = np.zeros((BLK, BLK), dtype=bool)
    keep = np.ones((BLK, BLK), dtype=bool)

    def keep_for_r(r):
        cols = []
        for cb in range(QTW // BLK):
            cols.append(full if cb < r else tri if cb == r else keep)
        return np.concatenate(cols, axis=1)  # [128, 512] bool

    kp = np.concatenate([keep_for_r(h), keep_for_r(h + 2)[:, QTW // 2 :]], axis=1)
    return np.where(kp, np.float16(SCHR_B), np.float16(-60000.0))


def kernel(query, key, value, padding):
    query = np.asarray(query, dtype=np.float32)
    key = np.asarray(key, dtype=np.float32)
    value = np.asarray(value, dtype=np.float32)
    padding = np.asarray(padding, dtype=bool)

    nc = _build_program()

    in_maps = []
    for c in range(NCORES):
        b, h = divmod(c, 2)
        qt1 = np.ascontiguousarray(query[b].T).astype(np.float16)  # [64, 4096]
        qt = np.concatenate([qt1, qt1], axis=0)  # [128, 4096] (row-tiling dup)
        kT = key[b].T  # [64, 4096] view
        blocks = [2 * i + h for i in range(PAR)]
        kt = np.concatenate([kT[:, BLK * j : BLK * (j + 1)] for j in blocks], axis=1)
        kt1 = np.ascontiguousarray(kt).astype(np.float16)  # [64, 2048]
        kt = np.concatenate([kt1, kt1], axis=0)  # [128, 2048] (row-tiling dup)
        vp = np.zeros((BLK, PAR * BLK), dtype=np.float16)
        for i, j in enumerate(blocks):
            vblk = value[b, BLK * j : BLK * (j + 1), :].copy()
            pblk = padding[b, BLK * j : BLK * (j + 1)]
            vblk[pblk] = 0.0
            # 1/16 scaling (exact power of two) keeps fp16 staging small;
            # cancels in num/den on the host.
            vp[:, BLK * i : BLK * i + 64] = vblk / 16.0
            vp[:, BLK * i + 64] = np.where(pblk, 0.0, 1.0 / 16.0)
        in_maps.append({"qt": qt, "kt": kt, "vp": vp, "mk": _make_maskbias(h)})

    global _last_in_maps
    _last_in_maps = in_maps
    res = run_bass_kernel_spmd(nc, in_maps, list(range(NCORES)))

    out = np.empty((B, S, D), dtype=np.float32)
    for b in range(B):
        r0 = res.results[2 * b]["out"].astype(np.float64)
        r1 = res.results[2 * b + 1]["out"].astype(np.float64)
        num = r0[:64] + r1[:64]  # [64, 4096]
        den = r0[64] + r1[64]  # [4096]
        out[b] = (num / den).T.astype(np.float32)
    return out


# revision 21
# speedup vs baseline: 1.1635x; 1.1415x over previous
"""Causal attention (B=4, S=4096, D=64, fp32) on 8 Trainium2 NeuronCores.

Strategy
--------
Sharding: 2 cores per batch element; the two cores of a batch split the KV
blocks by parity (even / odd 128-row blocks). Each core computes, for every
query position of its batch, the *unnormalized* attention numerator and the
softmax denominator contribution of its own KV half. The host sums the two
halves and divides (exactly linear: no max-subtraction; scores/8 are ~N(0,1)
so exp(s/8) <= ~1.7e3 stays in fp16 range; V and the denominator ones-column
are pre-scaled by 1/16 on the host - an exact power of two that cancels in
num/den - to keep the fp16 staging small).

Per-core device kernel (identical SPMD program; per-core behavior comes only
from input data), per 512-wide q tile, per kv block *pair*:
  - scores^T: S_T[kv, q] = K @ Q^T in fp16, two row-tiled matmuls (Q^T/K^T
    duplicated onto partitions 64-127 so the pair runs concurrently in the
    128x128 PE array), fp32 PSUM [128, 1024] (boundary pairs: 768 wide).
  - P = exp(s/8) in fp16, produced by TWO engines in parallel (the kernel is
    exp-throughput-bound; ACT alone was the baseline bottleneck):
      * ACT: activation(Exp, scale=0.125), PSUM -> fp16 SBUF,
        ~(w+172)/1.2GHz per pair.
      * DVE: one tensor_scalar op computing the Schraudolph bit trick
        bits = rne(s*(1024*log2e/8) + 15316) -> int16 SBUF, bitcast fp16
        (~+-3% sawtooth error; washes out after softmax normalization to
        ~4e-3 max rel err vs the 2e-2 budget), ~(w+~385)/0.96GHz.
    Pairs (and the per-tile output copies) are greedily load-balanced
    between the two engines with measured per-op costs.
  - causal masking: only the tile's diagonal (boundary) pair needs it; 0/1
    multiplicative masks are applied to P *after* exp on the otherwise-idle
    GPSIMD engine (keeps both exp engines and the PE mask-free). Boundary
    pairs run FIRST within their tile so the gpsimd latency hides behind the
    other pairs' exp stream (except the very first tile, where there is no
    backlog yet - there they run last).
  - numerator+denominator: matmul(lhsT=[V/16 | 1/16] block [128,65], rhs=P
    block [128,512]) accumulated over kv blocks in PSUM; row 64 is the
    softmax denominator. Padded keys: V rows and ones entries zeroed on host.
    mm2s are emitted TWO pairs behind their exp (software pipelining) so the
    in-order PE queue never parks on an exp semaphore while mm1 work exists -
    PE idle gaps would re-arm the HAM clock throttle and halve the PE clock.
  - output: [65, 512] PSUM -> fp16 SBUF copy (engine greedy) -> DMA.
Host: transposes Q/K, packs per-core inputs, combines/normalizes/transposes.
"""

import numpy as np
from contextlib import ExitStack

import concourse.tile as tile
from concourse import bacc, mybir
from concourse.bass_utils import run_bass_kernel_spmd

B, S, D = 4, 4096, 64
NCORES = 8
BLK = 128            # kv block rows
QTW = 512            # q tile width
NQT = S // QTW       # 8 q tiles
PAR = S // BLK // 2  # 16 kv blocks per parity half
WARMUP_MMS = 9       # dummy matmuls to open the PE HAM clock gate at startup
DEFER = 3            # pairs of software-pipelining between exp and mm2

LOG2E = float(np.log2(np.e))
SCHR_A = 1024.0 * LOG2E / 8.0   # d(bits)/d(score) for fp16 exp(s/8)
SCHR_B = 15360.0 - 44.0         # fp16 exponent bias + sawtooth centering

# Measured per-op engine costs (ns) for the greedy balance.
COST_ACT = {1024: 997.0, 768: 783.0}
COST_DVE = {1024: 1468.0, 768: 1136.0}
COST_STT = 1100.0    # boundary scalar_tensor_tensor (768 wide) on DVE
COST_COPY = 1110.0   # merged [65, 1024] fp32->fp16 PSUM->SBUF copy
# Tiles are processed in this order; consecutive tiles SHARE one [65, 1024]
# PSUM accumulator (out_ps bufs=1) so there are 4 output copies, not 8.
TILE_ORDER = [7, 0, 6, 5, 4, 3, 2, 1]

_prog_cache = {}


def _schedule():
    """[(T, p, boundary, engine)] in processing order + per-group copy engine.
    engine/copy: 0 = ACT, 1 = DVE. Groups are TILE_ORDER[2g:2g+2]."""
    seq = []
    for ti, T in enumerate(TILE_ORDER):
        npair = T + 1
        body = list(range(npair - 1))
        pairs = body + [npair - 1] if ti == 0 else [npair - 1] + body
        for p in pairs:
            seq.append((T, p, p == npair - 1))
    t_eng = [0.0, 0.0]
    out = []
    copy_eng = {}
    done_pairs = {T: 0 for T in TILE_ORDER}
    group_of = {T: gi // 2 for gi, T in enumerate(TILE_ORDER)}
    group_left = {g: 0 for g in range(len(TILE_ORDER) // 2)}
    for T in TILE_ORDER:
        group_left[group_of[T]] += T + 1
    for T, p, bnd in seq:
        if bnd:
            # boundary pairs carry the causal mask fused into the DVE
            # scalar_tensor_tensor op - DVE only
            t_eng[1] += COST_STT
            out.append((T, p, bnd, 1))
            done_pairs[T] += 1
            group_left[group_of[T]] -= 1
            if group_left[group_of[T]] == 0:
                e = 0 if t_eng[0] <= t_eng[1] else 1
                t_eng[e] += COST_COPY
                copy_eng[group_of[T]] = e
            continue
        ca, cv = COST_ACT[1024], COST_DVE[1024]
        if t_eng[0] + ca <= t_eng[1] + cv:
            t_eng[0] += ca
            out.append((T, p, bnd, 0))
        else:
            t_eng[1] += cv
            out.append((T, p, bnd, 1))
        done_pairs[T] += 1
        group_left[group_of[T]] -= 1
        if group_left[group_of[T]] == 0:  # group complete -> assign its copy
            e = 0 if t_eng[0] <= t_eng[1] else 1
            t_eng[e] += COST_COPY
            copy_eng[group_of[T]] = e
    return out, copy_eng


def _build_program():
    if "nc" in _prog_cache:
        return _prog_cache["nc"]
    nc = bacc.Bacc("TRN2", target_bir_lowering=False, debug=False, num_devices=NCORES)
    f32, f16, i16 = mybir.dt.float32, mybir.dt.float16, mybir.dt.int16
    Exp = mybir.ActivationFunctionType.Exp
    TWO = QTW * 2

    qt_d = nc.dram_tensor("qt", [2 * D, S], f16, kind="ExternalInput").ap()
    kt_d = nc.dram_tensor("kt", [2 * D, PAR * BLK], f16, kind="ExternalInput").ap()
    vp_d = nc.dram_tensor("vp", [BLK, PAR * BLK], f16, kind="ExternalInput").ap()
    mk_d = nc.dram_tensor("mk", [BLK, QTW + QTW // 2], f16, kind="ExternalInput").ap()
    out_d = nc.dram_tensor("out", [65, S], f16, kind="ExternalOutput").ap()

    sched, copy_eng = _schedule()

    with tile.TileContext(nc) as tc, ExitStack() as ctx:
        const = ctx.enter_context(tc.tile_pool(name="const", bufs=1))
        pa_pool = ctx.enter_context(tc.tile_pool(name="pa", bufs=5))
        pv_pool = ctx.enter_context(tc.tile_pool(name="pv", bufs=5))
        opool = ctx.enter_context(tc.tile_pool(name="op", bufs=2))
        sc_ps = ctx.enter_context(tc.tile_pool(name="scps", bufs=3, space="PSUM"))
        out_ps = ctx.enter_context(tc.tile_pool(name="ops", bufs=1, space="PSUM"))

        # Input DMAs spread over three rings (sync HWDGE, scalar HWDGE,
        # gpsimd SWDGE) in first-use order (tile 7, kv pairs ascending).
        mk_s = const.tile([BLK, QTW + QTW // 2], f16)
        kt_s = const.tile([2 * D, PAR * BLK], f16)
        vp_s = const.tile([BLK, PAR * BLK], f16)
        qt_s = const.tile([2 * D, S], f16)
        nc.scalar.dma_start(kt_s[:, 0:256], kt_d[:, 0:256])
        nc.gpsimd.dma_start(vp_s[:], vp_d[:])
        nc.scalar.dma_start(kt_s[:, 256:512], kt_d[:, 256:512])
        nc.scalar.dma_start(kt_s[:, 512:1024], kt_d[:, 512:1024])
        nc.scalar.dma_start(kt_s[:, 1024:1536], kt_d[:, 1024:1536])
        nc.scalar.dma_start(kt_s[:, 1536:], kt_d[:, 1536:])
        nc.gpsimd.dma_start(mk_s[:], mk_d[:])
        for t in [7, 0, 6, 5, 4, 3, 2, 1]:  # matches tile processing order
            nc.sync.dma_start(qt_s[:, t * QTW : (t + 1) * QTW], qt_d[:, t * QTW : (t + 1) * QTW])

        # PE warmup: dependency-free dummy matmuls during the preamble/DMA
        # window so the HAM clock gate (PE parked at 1.2 GHz until ~3.4us of
        # busy) opens before the first real matmul.
        wsrc = const.tile([BLK, QTW], f16, name="wsrc")
        nc.vector.memset(wsrc[:], 0.0)
        wps = sc_ps.tile([BLK, TWO], f32, tag="sc", name="wps")
        for _ in range(WARMUP_MMS):
            nc.tensor.matmul(wps[:, 0:QTW], wsrc[:, 0:BLK], wsrc[:], start=True, stop=True)

        # ---- software-pipelined main loop ----
        group_of = {T: gi // 2 for gi, T in enumerate(TILE_ORDER)}
        col_of = {T: (gi % 2) * QTW for gi, T in enumerate(TILE_ORDER)}
        gstate = {}   # group -> dict(ops=..., n per tile)
        pending = []  # deferred mm2 work items

        last_group = len(TILE_ORDER) // 2 - 1

        def emit_mm2(item):
            T, p, boundary, pt, wid = item
            g = group_of[T]
            st = gstate[g]
            depth = 2 * (T + 1)
            off = col_of[T]
            for k in (0, 1):
                blk = 2 * p + k
                st[T] += 1
                nc.tensor.matmul(
                    st["ops"][0:BLK, off + QTW - wid[k] : off + QTW],
                    vp_s[:, blk * BLK : blk * BLK + BLK],
                    pt[0:BLK, k * QTW : k * QTW + wid[k]],
                    start=(st[T] == 1),
                    stop=(st[T] == depth),
                )
            st["left"] -= 1
            if g == last_group:
                # tail: copy/DMA each tile as soon as it completes, so only
                # the final tile's [65, 512] chain trails the last exp.
                if st[T] == depth:
                    osb = opool.tile([65, QTW], f16, tag="osb", name=f"osbt{T}")
                    nc.scalar.copy(osb[:], st["ops"][0:65, col_of[T] : col_of[T] + QTW])
                    nc.scalar.dma_start(out_d[:, T * QTW : (T + 1) * QTW], osb[:])
            elif st["left"] == 0:
                osb = opool.tile([65, TWO], f16, tag="osb", name=f"osb{g}")
                if copy_eng[g] == 0:
                    nc.scalar.copy(osb[:], st["ops"][0:65, :])
                else:
                    nc.vector.tensor_copy(osb[:], st["ops"][0:65, :])
                for TT in TILE_ORDER[2 * g : 2 * g + 2]:
                    nc.sync.dma_start(
                        out_d[:, TT * QTW : (TT + 1) * QTW],
                        osb[:, col_of[TT] : col_of[TT] + QTW],
                    )

        # Pairs are processed in batches of 2: both pairs' mm1s are emitted
        # back-to-back (row-tiled PE config), then the deferred mm2s (full
        # 128x128 config) - one config switch per direction per batch instead
        # of per pair. Each switch exposes one ~100ns weight load.
        for ci in range(0, len(sched), 2):
            chunk = sched[ci : ci + 2]
            exps = []
            for T, p, boundary, eng in chunk:
                g = group_of[T]
                if g not in gstate:
                    gstate[g] = {
                        "ops": out_ps.tile([BLK, TWO], f32, tag="ops", name=f"ops{g}"),
                        "left": sum(TT + 1 for TT in TILE_ORDER[2 * g : 2 * g + 2]),
                    }
                    for TT in TILE_ORDER[2 * g : 2 * g + 2]:
                        gstate[g][TT] = 0
                sc = sc_ps.tile([BLK, TWO], f32, tag="sc")
                wid = (QTW, QTW // 2) if boundary else (QTW, QTW)
                for k, rg in ((0, 0), (1, D)):  # row group 0 / 64 (row tiling)
                    blk = 2 * p + k
                    nc.tensor.matmul(
                        sc[:, k * QTW : k * QTW + wid[k]],
                        kt_s[rg : rg + D, blk * BLK : (blk + 1) * BLK],
                        qt_s[rg : rg + D, (T + 1) * QTW - wid[k] : (T + 1) * QTW],
                        start=True,
                        stop=True,
                        tile_position=(rg, 0),
                    )
                exps.append((T, p, boundary, eng, sc, wid))
            for T, p, boundary, eng, sc, wid in exps:
                ew = QTW + wid[1]
                if boundary:
                    # DVE fast-exp with the causal mask FUSED as an additive
                    # bias tensor: bits = rne(s*A + mb) -> int16, bitcast
                    # fp16. Masked: mb=-60000 saturates to -32768 = -0.0.
                    pt_raw = pv_pool.tile([BLK, TWO], i16, tag="pv", name="ptv")
                    nc.vector.scalar_tensor_tensor(
                        pt_raw[:, 0:ew],
                        sc[:, 0:ew],
                        SCHR_A,
                        mk_s[:, 0:ew],
                        mybir.AluOpType.mult,
                        mybir.AluOpType.add,
                    )
                    pt = pt_raw[:].bitcast(f16)
                elif eng == 1:
                    # DVE fast-exp: bits = rne(s*A+B) -> int16, bitcast fp16.
                    pt_raw = pv_pool.tile([BLK, TWO], i16, tag="pv", name="ptv")
                    nc.vector.tensor_scalar(
                        pt_raw[:, 0:ew],
                        sc[:, 0:ew],
                        SCHR_A,
                        SCHR_B,
                        mybir.AluOpType.mult,
                        mybir.AluOpType.add,
                    )
                    pt = pt_raw[:].bitcast(f16)
                else:
                    pt_raw = pa_pool.tile([BLK, TWO], f16, tag="pa", name="pta")
                    pt = pt_raw[:]
                    nc.scalar.activation(pt[0:BLK, 0:ew], sc[:, 0:ew], Exp, scale=0.125)
                pending.append((T, p, boundary, pt, wid))
            while len(pending) > DEFER:
                emit_mm2(pending.pop(0))
        while pending:
            emit_mm2(pending.pop(0))

    nc.compile()
    _prog_cache["nc"] = nc
    return nc


def _make_maskbias(h):
    """[128, 768] fp16 additive Schraudolph bias for the boundary pair:
    +B where kept, -60000 where masked (saturates the int16 convert to
    -32768 = fp16 -0.0). Cols 0:512 = lo block (relative diagonal offset
    r = h); cols 512:768 = the computed 256-col slice of the hi block
    (r = h + 2, its q cols 256:512)."""
    tri = np.arange(QTW)[None, :BLK] >= np.arange(BLK)[:, None]
    full = np.zeros((BLK, BLK), dtype=bool)
    keep = np.ones((BLK, BLK), dtype=bool)

    def keep_for_r(r):
        cols = []
        for cb in range(QTW // BLK):
            cols.append(full if cb < r else tri if cb == r else keep)
        return np.concatenate(cols, axis=1)  # [128, 512] bool

    kp = np.concatenate([keep_for_r(h), keep_for_r(h + 2)[:, QTW // 2 :]], axis=1)
    return np.where(kp, np.float16(SCHR_B), np.float16(-60000.0))


def kernel(query, key, value, padding):
    query = np.asarray(query, dtype=np.float32)
    key = np.asarray(key, dtype=np.float32)
    value = np.asarray(value, dtype=np.float32)
    padding = np.asarray(padding, dtype=bool)

    nc = _build_program()

    in_maps = []
    for c in range(NCORES):
        b, h = divmod(c, 2)
        qt1 = np.ascontiguousarray(query[b].T).astype(np.float16)  # [64, 4096]
        qt = np.concatenate([qt1, qt1], axis=0)  # [128, 4096] (row-tiling dup)
        kT = key[b].T  # [64, 4096] view
        blocks = [2 * i + h for i in range(PAR)]
        kt = np.concatenate([kT[:, BLK * j : BLK * (j + 1)] for j in blocks], axis=1)
        kt1 = np.ascontiguousarray(kt).astype(np.float16)  # [64, 2048]
        kt = np.concatenate([kt1, kt1], axis=0)  # [128, 2048] (row-tiling dup)
        vp = np.zeros((BLK, PAR * BLK), dtype=np.float16)
        for i, j in enumerate(blocks):
            vblk = value[b, BLK * j : BLK * (j + 1), :].copy()
            pblk = padding[b, BLK * j : BLK * (j + 1)]
            vblk[pblk] = 0.0
            # 1/16 scaling (exact power of two) keeps fp16 staging small;
            # cancels in num/den on the host.
            vp[:, BLK * i : BLK * i + 64] = vblk / 16.0
            vp[:, BLK * i + 64] = np.where(pblk, 0.0, 1.0 / 16.0)
        in_maps.append({"qt": qt, "kt": kt, "vp": vp, "mk": _make_maskbias(h)})

    global _last_in_maps
    _last_in_maps = in_maps
    res = run_bass_kernel_spmd(nc, in_maps, list(range(NCORES)))

    out = np.empty((B, S, D), dtype=np.float32)
    for b in range(B):
        r0 = res.results[2 * b]["out"].astype(np.float64)
        r1 = res.results[2 * b + 1]["out"].astype(np.float64)
        num = r0[:64] + r1[:64]  # [64, 4096]
        den = r0[64] + r1[64]  # [4096]
        out[b] = (num / den).T.astype(np.float32)
    return out


# revision 22
# speedup vs baseline: 1.2170x; 1.0460x over previous
"""Causal attention (B=4, S=4096, D=64, fp32) on 8 Trainium2 NeuronCores.

Strategy
--------
Sharding: 2 cores per batch element; the two cores of a batch split the KV
blocks by parity (even / odd 128-row blocks). Each core computes, for every
query position of its batch, the *unnormalized* attention numerator and the
softmax denominator contribution of its own KV half. The host sums the two
halves and divides (exactly linear: no max-subtraction; scores/8 are ~N(0,1)
so exp(s/8) <= ~1.7e3 stays in fp16 range; V and the denominator ones-column
are pre-scaled by 1/16 on the host - an exact power of two that cancels in
num/den - to keep the fp16 staging small).

Per-core device kernel (identical SPMD program; per-core behavior comes only
from input data), per 512-wide q tile, per kv block *pair*:
  - scores^T: S_T[kv, q] = K @ Q^T in fp16, two row-tiled matmuls (Q^T/K^T
    duplicated onto partitions 64-127 so the pair runs concurrently in the
    128x128 PE array), fp32 PSUM [128, 1024] (boundary pairs: 768 wide).
  - P = exp(s/8) in fp16, produced by TWO engines in parallel (the kernel is
    exp-throughput-bound; ACT alone was the baseline bottleneck):
      * ACT: activation(Exp, scale=0.125), PSUM -> fp16 SBUF,
        ~(w+172)/1.2GHz per pair.
      * DVE: one tensor_scalar op computing the Schraudolph bit trick
        bits = rne(s*(1024*log2e/8) + 15316) -> int16 SBUF, bitcast fp16
        (~+-3% sawtooth error; washes out after softmax normalization to
        ~4e-3 max rel err vs the 2e-2 budget), ~(w+~385)/0.96GHz.
    Pairs (and the per-tile output copies) are greedily load-balanced
    between the two engines with measured per-op costs.
  - causal masking: only the tile's diagonal (boundary) pair needs it; 0/1
    multiplicative masks are applied to P *after* exp on the otherwise-idle
    GPSIMD engine (keeps both exp engines and the PE mask-free). Boundary
    pairs run FIRST within their tile so the gpsimd latency hides behind the
    other pairs' exp stream (except the very first tile, where there is no
    backlog yet - there they run last).
  - numerator+denominator: matmul(lhsT=[V/16 | 1/16] block [128,65], rhs=P
    block [128,512]) accumulated over kv blocks in PSUM; row 64 is the
    softmax denominator. Padded keys: V rows and ones entries zeroed on host.
    mm2s are emitted TWO pairs behind their exp (software pipelining) so the
    in-order PE queue never parks on an exp semaphore while mm1 work exists -
    PE idle gaps would re-arm the HAM clock throttle and halve the PE clock.
  - output: [65, 512] PSUM -> fp16 SBUF copy (engine greedy) -> DMA.
Host: transposes Q/K, packs per-core inputs, combines/normalizes/transposes.
"""

import numpy as np
from contextlib import ExitStack

import concourse.tile as tile
from concourse import bacc, mybir
from concourse.bass_utils import run_bass_kernel_spmd

B, S, D = 4, 4096, 64
NCORES = 8
BLK = 128            # kv block rows
QTW = 512            # q tile width
NQT = S // QTW       # 8 q tiles
PAR = S // BLK // 2  # 16 kv blocks per parity half
WARMUP_MMS = 9       # dummy matmuls to open the PE HAM clock gate at startup
DEFER = 3            # pairs of software-pipelining between exp and mm2

LOG2E = float(np.log2(np.e))
SCHR_A = 1024.0 * LOG2E / 8.0   # d(bits)/d(score) for fp16 exp(s/8)
SCHR_B = 15360.0 - 44.0         # fp16 exponent bias + sawtooth centering

# Measured per-op engine costs (ns) for the greedy balance.
COST_ACT = {1024: 997.0, 768: 783.0}
COST_DVE = {1024: 1468.0, 768: 1136.0}
COST_STT = 1100.0    # boundary scalar_tensor_tensor (768 wide) on DVE
COST_COPY = 1110.0   # merged [65, 1024] fp32->fp16 PSUM->SBUF copy
# Tiles are processed in this order; consecutive tiles SHARE one [65, 1024]
# PSUM accumulator (out_ps bufs=1) so there are 4 output copies, not 8.
TILE_ORDER = [7, 0, 6, 5, 4, 3, 2, 1]

_prog_cache = {}


def _schedule():
    """[(T, p, boundary, engine)] in processing order + per-group copy engine.
    engine/copy: 0 = ACT, 1 = DVE. Groups are TILE_ORDER[2g:2g+2]."""
    seq = []
    for ti, T in enumerate(TILE_ORDER):
        npair = T + 1
        body = list(range(npair - 1))
        pairs = body + [npair - 1]
        for p in pairs:
            seq.append((T, p, p == npair - 1))
    t_eng = [0.0, 0.0]
    out = []
    copy_eng = {}
    done_pairs = {T: 0 for T in TILE_ORDER}
    group_of = {T: gi // 2 for gi, T in enumerate(TILE_ORDER)}
    group_left = {g: 0 for g in range(len(TILE_ORDER) // 2)}
    for T in TILE_ORDER:
        group_left[group_of[T]] += T + 1
    for T, p, bnd in seq:
        if bnd:
            # boundary pairs carry the causal mask fused into the DVE
            # scalar_tensor_tensor op - DVE only
            t_eng[1] += COST_STT
            out.append((T, p, bnd, 1))
            done_pairs[T] += 1
            group_left[group_of[T]] -= 1
            if group_left[group_of[T]] == 0:
                e = 0 if t_eng[0] <= t_eng[1] else 1
                t_eng[e] += COST_COPY
                copy_eng[group_of[T]] = e
            continue
        ca, cv = COST_ACT[1024], COST_DVE[1024]
        if t_eng[0] + ca <= t_eng[1] + cv:
            t_eng[0] += ca
            out.append((T, p, bnd, 0))
        else:
            t_eng[1] += cv
            out.append((T, p, bnd, 1))
        done_pairs[T] += 1
        group_left[group_of[T]] -= 1
        if group_left[group_of[T]] == 0:  # group complete -> assign its copy
            e = 0 if t_eng[0] <= t_eng[1] else 1
            t_eng[e] += COST_COPY
            copy_eng[group_of[T]] = e
    return out, copy_eng


def _build_program():
    if "nc" in _prog_cache:
        return _prog_cache["nc"]
    nc = bacc.Bacc("TRN2", target_bir_lowering=False, debug=False, num_devices=NCORES)
    f32, f16, i16 = mybir.dt.float32, mybir.dt.float16, mybir.dt.int16
    Exp = mybir.ActivationFunctionType.Exp
    TWO = QTW * 2

    qt_d = nc.dram_tensor("qt", [2 * D, S], f16, kind="ExternalInput").ap()
    kt_d = nc.dram_tensor("kt", [2 * D, PAR * BLK], f16, kind="ExternalInput").ap()
    vp_d = nc.dram_tensor("vp", [BLK, PAR * BLK], f16, kind="ExternalInput").ap()
    mk_d = nc.dram_tensor("mk", [BLK, QTW + QTW // 2], f16, kind="ExternalInput").ap()
    out_d = nc.dram_tensor("out", [65, S], f16, kind="ExternalOutput").ap()

    sched, copy_eng = _schedule()

    with tile.TileContext(nc) as tc, ExitStack() as ctx:
        const = ctx.enter_context(tc.tile_pool(name="const", bufs=1))
        pa_pool = ctx.enter_context(tc.tile_pool(name="pa", bufs=5))
        pv_pool = ctx.enter_context(tc.tile_pool(name="pv", bufs=5))
        opool = ctx.enter_context(tc.tile_pool(name="op", bufs=2))
        sc_ps = ctx.enter_context(tc.tile_pool(name="scps", bufs=3, space="PSUM"))
        out_ps = ctx.enter_context(tc.tile_pool(name="ops", bufs=1, space="PSUM"))

        # Input DMAs spread over three rings (sync HWDGE, scalar HWDGE,
        # gpsimd SWDGE) in first-use order (tile 7, kv pairs ascending).
        mk_s = const.tile([BLK, QTW + QTW // 2], f16)
        kt_s = const.tile([2 * D, PAR * BLK], f16)
        vp_s = const.tile([BLK, PAR * BLK], f16)
        qt_s = const.tile([2 * D, S], f16)
        nc.scalar.dma_start(kt_s[:, 0:256], kt_d[:, 0:256])
        nc.gpsimd.dma_start(vp_s[:], vp_d[:])
        nc.scalar.dma_start(kt_s[:, 256:512], kt_d[:, 256:512])
        nc.scalar.dma_start(kt_s[:, 512:1024], kt_d[:, 512:1024])
        nc.scalar.dma_start(kt_s[:, 1024:1536], kt_d[:, 1024:1536])
        nc.scalar.dma_start(kt_s[:, 1536:], kt_d[:, 1536:])
        nc.gpsimd.dma_start(mk_s[:], mk_d[:])
        for t in [7, 0, 6, 5, 4, 3, 2, 1]:  # matches tile processing order
            nc.sync.dma_start(qt_s[:, t * QTW : (t + 1) * QTW], qt_d[:, t * QTW : (t + 1) * QTW])

        # PE warmup: dependency-free dummy matmuls during the preamble/DMA
        # window so the HAM clock gate (PE parked at 1.2 GHz until ~3.4us of
        # busy) opens before the first real matmul.
        wsrc = const.tile([BLK, QTW], f16, name="wsrc")
        nc.vector.memset(wsrc[:], 0.0)
        wps = sc_ps.tile([BLK, TWO], f32, tag="sc", name="wps")
        for _ in range(WARMUP_MMS):
            nc.tensor.matmul(wps[:, 0:QTW], wsrc[:, 0:BLK], wsrc[:], start=True, stop=True)

        # ---- software-pipelined main loop ----
        group_of = {T: gi // 2 for gi, T in enumerate(TILE_ORDER)}
        col_of = {T: (gi % 2) * QTW for gi, T in enumerate(TILE_ORDER)}
        gstate = {}   # group -> dict(ops=..., n per tile)
        pending = []  # deferred mm2 work items

        last_group = len(TILE_ORDER) // 2 - 1

        def emit_mm2(item):
            T, p, boundary, pt, wid = item
            g = group_of[T]
            st = gstate[g]
            depth = 2 * (T + 1)
            off = col_of[T]
            for k in (0, 1):
                blk = 2 * p + k
                st[T] += 1
                nc.tensor.matmul(
                    st["ops"][0:BLK, off + QTW - wid[k] : off + QTW],
                    vp_s[:, blk * BLK : blk * BLK + BLK],
                    pt[0:BLK, k * QTW : k * QTW + wid[k]],
                    start=(st[T] == 1),
                    stop=(st[T] == depth),
                )
            st["left"] -= 1
            if g == last_group:
                # tail: copy/DMA each tile as soon as it completes, so only
                # the final tile's [65, 512] chain trails the last exp.
                if st[T] == depth:
                    osb = opool.tile([65, QTW], f16, tag="osb", name=f"osbt{T}")
                    nc.scalar.copy(osb[:], st["ops"][0:65, col_of[T] : col_of[T] + QTW])
                    nc.scalar.dma_start(out_d[:, T * QTW : (T + 1) * QTW], osb[:])
            elif st["left"] == 0:
                osb = opool.tile([65, TWO], f16, tag="osb", name=f"osb{g}")
                if copy_eng[g] == 0:
                    nc.scalar.copy(osb[:], st["ops"][0:65, :])
                else:
                    nc.vector.tensor_copy(osb[:], st["ops"][0:65, :])
                for TT in TILE_ORDER[2 * g : 2 * g + 2]:
                    nc.sync.dma_start(
                        out_d[:, TT * QTW : (TT + 1) * QTW],
                        osb[:, col_of[TT] : col_of[TT] + QTW],
                    )

        # Pairs are processed in batches of 2: both pairs' mm1s are emitted
        # back-to-back (row-tiled PE config), then the deferred mm2s (full
        # 128x128 config) - one config switch per direction per batch instead
        # of per pair. Each switch exposes one ~100ns weight load.
        for ci in range(0, len(sched), 2):
            chunk = sched[ci : ci + 2]
            exps = []
            for T, p, boundary, eng in chunk:
                g = group_of[T]
                if g not in gstate:
                    gstate[g] = {
                        "ops": out_ps.tile([BLK, TWO], f32, tag="ops", name=f"ops{g}"),
                        "left": sum(TT + 1 for TT in TILE_ORDER[2 * g : 2 * g + 2]),
                    }
                    for TT in TILE_ORDER[2 * g : 2 * g + 2]:
                        gstate[g][TT] = 0
                sc = sc_ps.tile([BLK, TWO], f32, tag="sc")
                wid = (QTW, QTW // 2) if boundary else (QTW, QTW)
                for k, rg in ((0, 0), (1, D)):  # row group 0 / 64 (row tiling)
                    blk = 2 * p + k
                    nc.tensor.matmul(
                        sc[:, k * QTW : k * QTW + wid[k]],
                        kt_s[rg : rg + D, blk * BLK : (blk + 1) * BLK],
                        qt_s[rg : rg + D, (T + 1) * QTW - wid[k] : (T + 1) * QTW],
                        start=True,
                        stop=True,
                        tile_position=(rg, 0),
                    )
                exps.append((T, p, boundary, eng, sc, wid))
            for T, p, boundary, eng, sc, wid in exps:
                ew = QTW + wid[1]
                if boundary:
                    # DVE fast-exp with the causal mask FUSED as an additive
                    # bias tensor: bits = rne(s*A + mb) -> int16, bitcast
                    # fp16. Masked: mb=-60000 saturates to -32768 = -0.0.
                    pt_raw = pv_pool.tile([BLK, TWO], i16, tag="pv", name="ptv")
                    nc.vector.scalar_tensor_tensor(
                        pt_raw[:, 0:ew],
                        sc[:, 0:ew],
                        SCHR_A,
                        mk_s[:, 0:ew],
                        mybir.AluOpType.mult,
                        mybir.AluOpType.add,
                    )
                    pt = pt_raw[:].bitcast(f16)
                elif eng == 1:
                    # DVE fast-exp: bits = rne(s*A+B) -> int16, bitcast fp16.
                    pt_raw = pv_pool.tile([BLK, TWO], i16, tag="pv", name="ptv")
                    nc.vector.tensor_scalar(
                        pt_raw[:, 0:ew],
                        sc[:, 0:ew],
                        SCHR_A,
                        SCHR_B,
                        mybir.AluOpType.mult,
                        mybir.AluOpType.add,
                    )
                    pt = pt_raw[:].bitcast(f16)
                else:
                    pt_raw = pa_pool.tile([BLK, TWO], f16, tag="pa", name="pta")
                    pt = pt_raw[:]
                    nc.scalar.activation(pt[0:BLK, 0:ew], sc[:, 0:ew], Exp, scale=0.125)
                pending.append((T, p, boundary, pt, wid))
            while len(pending) > DEFER:
                emit_mm2(pending.pop(0))
        while pending:
            emit_mm2(pending.pop(0))

    nc.compile()
    _prog_cache["nc"] = nc
    return nc


def _make_maskbias(h):
    """[128, 768] fp16 additive Schraudolph bias for the boundary pair:
    +B where kept, -60000 where masked (saturates the int16 convert to
    -32768 = fp16 -0.0). Cols 0:512 = lo block (relative diagonal offset
    r = h); cols 512:768 = the computed 256-col slice of the hi block
    (r = h + 2, its q cols 256:512)."""
    tri = np.arange(QTW)[None, :BLK] >= np.arange(BLK)[:, None]
    full = np.zeros((BLK, BLK), dtype=bool)
    keep = np.ones((BLK, BLK), dtype=bool)

    def keep_for_r(r):
        cols = []
        for cb in range(QTW // BLK):
            cols.append(full if cb < r else tri if cb == r else keep)
        return np.concatenate(cols, axis=1)  # [128, 512] bool

    kp = np.concatenate([keep_for_r(h), keep_for_r(h + 2)[:, QTW // 2 :]], axis=1)
    return np.where(kp, np.float16(SCHR_B), np.float16(-60000.0))


def kernel(query, key, value, padding):
    query = np.asarray(query, dtype=np.float32)
    key = np.asarray(key, dtype=np.float32)
    value = np.asarray(value, dtype=np.float32)
    padding = np.asarray(padding, dtype=bool)

    nc = _build_program()

    in_maps = []
    for c in range(NCORES):
        b, h = divmod(c, 2)
        qt1 = np.ascontiguousarray(query[b].T).astype(np.float16)  # [64, 4096]
        qt = np.concatenate([qt1, qt1], axis=0)  # [128, 4096] (row-tiling dup)
        kT = key[b].T  # [64, 4096] view
        blocks = [2 * i + h for i in range(PAR)]
        kt = np.concatenate([kT[:, BLK * j : BLK * (j + 1)] for j in blocks], axis=1)
        kt1 = np.ascontiguousarray(kt).astype(np.float16)  # [64, 2048]
        kt = np.concatenate([kt1, kt1], axis=0)  # [128, 2048] (row-tiling dup)
        vp = np.zeros((BLK, PAR * BLK), dtype=np.float16)
        for i, j in enumerate(blocks):
            vblk = value[b, BLK * j : BLK * (j + 1), :].copy()
            pblk = padding[b, BLK * j : BLK * (j + 1)]
            vblk[pblk] = 0.0
            # 1/16 scaling (exact power of two) keeps fp16 staging small;
            # cancels in num/den on the host.
            vp[:, BLK * i : BLK * i + 64] = vblk / 16.0
            vp[:, BLK * i + 64] = np.where(pblk, 0.0, 1.0 / 16.0)
        in_maps.append({"qt": qt, "kt": kt, "vp": vp, "mk": _make_maskbias(h)})

    global _last_in_maps
    _last_in_maps = in_maps
    res = run_bass_kernel_spmd(nc, in_maps, list(range(NCORES)))

    out = np.empty((B, S, D), dtype=np.float32)
    for b in range(B):
        r0 = res.results[2 * b]["out"].astype(np.float64)
        r1 = res.results[2 * b + 1]["out"].astype(np.float64)
        num = r0[:64] + r1[:64]  # [64, 4096]
        den = r0[64] + r1[64]  # [4096]
        out[b] = (num / den).T.astype(np.float32)
    return out
